# revision 1
# baseline (speedup 1.0000x reference)
"""Trainium2 Bass kernel for nn_CalculateSLayer (GNN message passing).

Computes, for adj (N, N, 2) f32 and s (N, D) f32:
    a     = adj.sum(axis=2)                  # (N, N)
    s_in  = a.T @ s                          # (N, D)
    s_out = a @ s                            # (N, D)
returns (s_in, s_out) — matching the reference's output tuple.

Distribution: adjacency is sharded row-wise across 8 NeuronCores.  Core c
owns rows I_c = [c*512, (c+1)*512).  From its (512, 4096, 2) block it
computes on-device:
  * a partial s_in^T (D, N)    = (s[I_c]).T @ a[I_c]       (contracts i)
  * its exact  s_out^T (D,512) from a[I_c].T               (contracts j)
The host sums the 8 s_in partials and concatenates the s_out blocks.

Per-core dataflow (pipelined under Tile/Bacc; ~47 us HBM roofline):
  DMA : all 32 (128 x 512j x 2k) raw loads issued up front — 4 KB/partition
        contiguous lines stream at HBM rate (~410 GB/s observed).
  DVE : channel-reduce a_ch[i, j] = raw[i, j, 0] + raw[i, j, 1]  (f32r out).
  PE  : s_in matmul  psum_sin(70,512) += s_own[it].T @ a_ch      (f32r moving)
        transposes   psT[t][j, it*128+i] = a_ch[i, t*128+j]      (via identity)
        s_out matmul psum_out(70,512) += s_all[jt].T @ aT[t]     (f32r moving),
        pipelined one chunk behind the transposes so the PE never stalls on
        PSUM evacuation; the final chunk runs at half-tile granularity so
        only its hi halves sit in the post-last-data tail.
  ACT : psT -> aT SBUF evacuation.   DVE: psum_sin -> SBUF staging.
  DMA : s_in^T flushed in four quarter DMAs; s_out^T at the end (SWDGE).

Notes:
  * float32r runs the PE at 1 cycle/row for moving free-dim >= 256 (plain
    f32 is 4 cycles/row); transposes use the exact permutation path; fp32r
    operand rounding happens in the producing DVE/ACT ops (walrus requires
    it).  Measured rel L2 error vs the f32 reference: 1.3e-4.
  * Persistent tiles (no pool slot churn) keep Tile's semaphore waits
    near-minimal; Bacc's event-semaphore pass absorbs the rest.
  * Measured ~71-74 us per core: ~7 us framework preamble + ~50 us DMA
    window (HBM-bound, shared-stack contention) + short compute tail +
    ~10 us Tile exit barrier.
"""

import numpy as np

import concourse.bass as bass
from concourse import bacc
import concourse.mybir as mybir
import concourse.tile as tile
from concourse import bass_utils

N = 4096          # nodes
D = 70            # embedding dim
NCORES = 8
RB = N // NCORES  # 512 rows per core
P = 128           # partitions
IT = RB // P      # 4 i-tiles per core
WJ = 512          # j-chunk width
JC = N // WJ      # 8 j-chunks
JT = WJ // P      # 4 transpose subtiles per chunk
NJT = N // P      # 32 s_all subtiles

F32 = mybir.dt.float32
F32R = mybir.dt.float32r

# Set by the test harness to capture a profile; the grading path leaves these
# untouched.
TRACE = False
TRACE_KWARGS = {}
LAST_RESULT = None


def _emit(nc: bass.Bass, adj_blk, s_own, s_all, s_inT, s_outT):
    with tile.TileContext(nc) as tc:
        with (
            # one buffer per (chunk, i-tile): no slot reuse, maximal prefetch
            tc.tile_pool(name="raw", bufs=JC * IT) as raw_pool,
            tc.tile_pool(name="work", bufs=1) as work,
            tc.tile_pool(name="singles", bufs=1) as singles,
            tc.tile_pool(name="psT", bufs=1, space="PSUM") as psT_pool,
            tc.tile_pool(name="psSin", bufs=1, space="PSUM") as psSin_pool,
            tc.tile_pool(name="psOut", bufs=1, space="PSUM") as psOut_pool,
        ):
            # (i_tile, partition) view of the raw block
            adj_r = adj_blk.rearrange("(t p) j k -> p t j k", p=P)

            # issue every raw load up front: per-(chunk, i-tile) granularity
            # so the first adds start as soon as 512 KB lands; the DMA queues
            # then stream the full 16.8 MB back-to-back at HBM rate
            raws = [[None] * IT for _ in range(JC)]
            for jc in range(JC):
                for it in range(IT):
                    r = raw_pool.tile([P, WJ, 2], F32, tag="raw")
                    nc.sync.dma_start(
                        out=r, in_=adj_r[:, it, jc * WJ : (jc + 1) * WJ, :]
                    )
                    raws[jc][it] = r
                if jc == 0:
                    # constants ride the DMA queue right after chunk 0
                    ident_dram = nc.inline_tensor(
                        np.eye(P, dtype=np.float32), name="ident_const"
                    )
                    ident = singles.tile([P, P], F32R)
                    nc.sync.dma_start(
                        out=ident, in_=ident_dram.ap().bitcast(F32R)
                    )
                    with tc.tile_pool(name="stage", bufs=1) as stage:
                        s_own_st = stage.tile([P, IT, D], F32)
                        nc.sync.dma_start(
                            out=s_own_st,
                            in_=s_own.rearrange("(t p) d -> p t d", p=P),
                        )
                        s_own_sb = singles.tile([P, IT, D], F32R)
                        nc.vector.tensor_copy(out=s_own_sb, in_=s_own_st)
                        s_all_st = stage.tile([P, NJT, D], F32)
                        nc.sync.dma_start(
                            out=s_all_st,
                            in_=s_all.rearrange("(t p) d -> p t d", p=P),
                        )
                        s_all_sb = singles.tile([P, NJT, D], F32R)
                        nc.vector.tensor_copy(out=s_all_sb, in_=s_all_st)

            # persistent working tiles
            a_chs = [
                [
                    work.tile([P, WJ], F32R, name=f"a_ch_{par}_{it}")
                    for it in range(IT)
                ]
                for par in range(2)
            ]
            aTs = [
                [work.tile([P, RB], F32R, name=f"aT_{par}_{t}") for t in range(JT)]
                for par in range(2)
            ]
            sin_sb_all = work.tile([D, N], F32, name="sin_sb_all")
            psT = [
                psT_pool.tile([P, RB], F32R, name=f"psT_{t}") for t in range(JT)
            ]
            psum_sins = [
                psSin_pool.tile([D, WJ], F32, name=f"psum_sin_{par}")
                for par in range(2)
            ]
            psum_out = psOut_pool.tile([D, RB], F32)

            def emit_sout_mm(jc, t):
                """One s_out^T accumulation for chunk jc, subtile t (aT
                already evacuated; runs one chunk behind so the PE never
                stalls on the PSUM->SBUF copies)."""
                jt = jc * JT + t
                nc.tensor.matmul(
                    psum_out,
                    lhsT=s_all_sb[:, jt, :],
                    rhs=aTs[jc % 2][t],
                    start=(jt == 0),
                    stop=(jt == NJT - 1),
                )

            for jc in range(JC):
                par = jc % 2
                psum_sin = psum_sins[par]
                for it in range(IT):
                    raw = raws[jc][it]
                    a_ch = a_chs[par][it]
                    nc.vector.tensor_add(
                        out=a_ch, in0=raw[:, :, 0], in1=raw[:, :, 1]
                    )
                    # a^T tiles: psT[t][j, it*128 + i] = a[i, t*128 + j]
                    for t in range(JT):
                        nc.tensor.transpose(
                            psT[t][:, it * P : (it + 1) * P],
                            a_ch[:, t * P : (t + 1) * P],
                            ident,
                        )
                    # s_in^T partial: psum_sin[d, j] += sum_i s_own[i, d]*a[i, j]
                    nc.tensor.matmul(
                        psum_sin,
                        lhsT=s_own_sb[:, it, :],
                        rhs=a_ch,
                        start=(it == 0),
                        stop=(it == IT - 1),
                    )
                    if it == IT - 1 and jc > 0:
                        # previous chunk's s_out matmuls: their aT operands
                        # finished copying while this chunk transposed
                        for t in range(JT):
                            emit_sout_mm(jc - 1, t)
                # evacuate s_in^T chunk (DVE) and a^T tiles (ACT), then flush
                nc.vector.tensor_copy(
                    out=sin_sb_all[:, jc * WJ : (jc + 1) * WJ], in_=psum_sin
                )
                if jc < JC - 1:
                    for t in range(JT):
                        nc.scalar.copy(out=aTs[par][t], in_=psT[t])
                if jc % 2 == 1:
                    q = jc // 2
                    nc.gpsimd.dma_start(
                        out=s_inT[q],
                        in_=sin_sb_all[:, q * (N // 4) : (q + 1) * (N // 4)],
                    )
            # epilogue: the final chunk's s_out path runs at half-tile
            # granularity so only the hi halves (i-tiles 2-3, transposed from
            # the last DMA chunks) sit in the post-last-data tail
            lpar = (JC - 1) % 2
            H = RB // 2
            for half in range(2):
                sl = slice(half * H, (half + 1) * H)
                for t in range(JT):
                    if half == 0:
                        nc.scalar.copy(out=aTs[lpar][t][:, sl], in_=psT[t][:, sl])
                    elif t % 2 == 1:
                        nc.vector.tensor_copy(
                            out=aTs[lpar][t][:, sl], in_=psT[t][:, sl]
                        )
                    else:
                        nc.scalar.copy(out=aTs[lpar][t][:, sl], in_=psT[t][:, sl])
                for t in range(JT):
                    jt = (JC - 1) * JT + t
                    nc.tensor.matmul(
                        psum_out[:, sl],
                        lhsT=s_all_sb[:, jt, :],
                        rhs=aTs[lpar][t][:, sl],
                        start=False,
                        stop=(jt == NJT - 1 and half == 1),
                    )
            s_outT_sb = singles.tile([D, RB], F32)
            nc.scalar.copy(out=s_outT_sb, in_=psum_out)
            nc.gpsimd.dma_start(out=s_outT, in_=s_outT_sb)


_ENGINE_SEM_PREFIX = {
    "PE": "PE_",
    "DVE": "DVE_",
    "Activation": "Activation_",
    "Pool": "Pool_",
    "SP": "SP_",
}

_SKIP_OPS = ("InstEventSemaphore", "InstDrain", "InstDMACopy", "InstBranch")


def _strip_self_waits(nc: bass.Bass) -> int:
    """Drop semaphore waits where an instruction waits on its OWN engine's
    completion semaphore.  Engine queues issue and complete in order, so such
    waits are always runtime-satisfied; Tile emits them anyway and they push
    instructions past walrus codegen's per-opcode sync-wait limits (most
    compute encodings accept a single wait)."""
    stripped = 0
    for _, inst in nc.inst_map.items():
        if type(inst).__name__ in _SKIP_OPS:
            continue
        si = getattr(inst, "sync_info", None)
        if si is None or not si.on_wait:
            continue
        eng = getattr(inst, "engine", None)
        prefix = _ENGINE_SEM_PREFIX.get(getattr(eng, "name", ""), None)
        if prefix is None:
            continue
        kept = [w for w in si.on_wait if not w.ant_name.startswith(prefix)]
        if len(kept) != len(si.on_wait):
            stripped += len(si.on_wait) - len(kept)
            si.on_wait = kept
    return stripped


def _build() -> bass.Bass:
    nc = bacc.Bacc("TRN2", num_devices=NCORES)
    adj_blk = nc.dram_tensor("adj_blk", [RB, N, 2], F32, kind="ExternalInput")
    s_own = nc.dram_tensor("s_own", [RB, D], F32, kind="ExternalInput")
    s_all = nc.dram_tensor("s_all", [N, D], F32, kind="ExternalInput")
    # one output tensor per j-chunk so the 8 output DMAs carry no cross-queue
    # write-ordering waits (HWDGE descriptors allow a single sync wait)
    s_inT = [
        nc.dram_tensor(f"s_inT_{h}", [D, N // 4], F32, kind="ExternalOutput")
        for h in range(4)
    ]
    s_outT = nc.dram_tensor("s_outT", [D, RB], F32, kind="ExternalOutput")
    _emit(
        nc,
        adj_blk.ap(),
        s_own.ap(),
        s_all.ap(),
        [t.ap() for t in s_inT],
        s_outT.ap(),
    )
    nc.finalize()
    return nc


_nc_cache = None


def kernel(adj: np.ndarray, s: np.ndarray):
    global _nc_cache, LAST_RESULT
    adj = np.ascontiguousarray(np.asarray(adj, dtype=np.float32))
    s = np.ascontiguousarray(np.asarray(s, dtype=np.float32))
    assert adj.shape == (N, N, 2) and s.shape == (N, D)

    if _nc_cache is None:
        _nc_cache = _build()
    nc = _nc_cache

    in_maps = [
        {
            "adj_blk": np.ascontiguousarray(adj[c * RB : (c + 1) * RB]),
            "s_own": np.ascontiguousarray(s[c * RB : (c + 1) * RB]),
            "s_all": s,
        }
        for c in range(NCORES)
    ]
    res = bass_utils.run_bass_kernel_spmd(
        nc,
        in_maps,
        core_ids=list(range(NCORES)),
        trace=TRACE,
        **TRACE_KWARGS,
    )
    LAST_RESULT = res

    s_in = (
        np.sum(
            [
                np.concatenate([r[f"s_inT_{h}"] for h in range(4)], axis=1)
                for r in res.results
            ],
            axis=0,
            dtype=np.float64,
        )
        .astype(np.float32)
        .T
    )
    s_out = np.concatenate([r["s_outT"].T for r in res.results], axis=0)
    return (np.ascontiguousarray(s_in), np.ascontiguousarray(s_out))



# revision 8
# speedup vs baseline: 1.6003x; 1.6003x over previous
"""Trainium2 Bass kernel for nn_CalculateSLayer (GNN message passing).

Computes, for adj (N, N, 2) f32 and s (N, D) f32:
    a     = adj.sum(axis=2)                  # (N, N)
    s_in  = a.T @ s                          # (N, D)
    s_out = a @ s                            # (N, D)
returns (s_in, s_out) — matching the reference's output tuple.

Distribution: adjacency rows sharded across 8 NeuronCores; core c owns
rows I_c = [c*512, (c+1)*512).  From its (512, 4096, 2) block it computes
  * a partial s_in^T (D, N)   = (s[I_c]).T @ a[I_c]   (contracts i)
  * its exact  s_out^T (D,512)  from a[I_c]^T         (contracts j)
Host sums the 8 s_in partials and concatenates the s_out blocks.

v2 (bf16): the kernel runs in bf16 (measured rel L2 error 2.6e-3 vs the
f32 reference — the threshold is 2e-2).  This halves the HBM stream
(16.8 -> 8.4 MB/core) and doubles PE matmul column rate.

Per-core dataflow:
  host: cast adj block to bf16 and relayout to [jc][p][k][it][j] so each
        chunk is one fully contiguous 1 MB DMA with 8 KB/partition lines
        (k-planes de-interleaved for contiguous DVE adds).  The last
        chunk is split into two 256-column subchunks to shorten the
        post-last-byte tail.
  DMA : 9 chunk loads issued up front on the sync HWDGE queue; small
        s/identity loads ride the gpsimd queue in parallel.
  DVE : channel add a_ch = k0 + k1 (bf16, contiguous), psum evacuations.
  PE  : ~20 warm-up matmuls on a zero tile defeat the HAM cold clock
        (1.2 GHz) before real work;
        pair-transposes: a_ch viewed as f32 packs two bf16 j's per
        element, so 128x128 f32 exact-permutation transposes move two
        j-columns at once (64 transposes instead of 128);
        s_in  matmul psum_sin(70,512) += s_own[it].T @ a_ch
        s_out matmul psum_out(70,512) += s_perm[jc,t,r].T @ aT[t][:,:,r]
        (aT viewed as [128, 512, 2] bf16; r indexes the packed pair),
        pipelined one chunk behind the transposes.
  ACT : psT -> aT SBUF evacuation (f32 bit-preserving).
  DMA : s_in^T quarter flushes (bf16) mid-stream, s_out^T at the end.
"""

import numpy as np
import ml_dtypes

import concourse.bass as bass
from concourse import bacc
import concourse.mybir as mybir
import concourse.tile as tile
from concourse import bass_utils

N = 4096          # nodes
D = 70            # embedding dim
NCORES = 8
RB = N // NCORES  # 512 rows per core
P = 128           # partitions
IT = RB // P      # 4 i-tiles per core
WJ = 512          # j-chunk width
JC = N // WJ      # 8 j-chunks
HJ = WJ // 2      # subchunk width for the last chunk
N_WARM = 12       # PE warm-up matmuls (HAM un-throttle)

F32 = mybir.dt.float32
BF16 = mybir.dt.bfloat16

# Set by the test harness to capture a profile; the grading path leaves these
# untouched.
TRACE = False
TRACE_KWARGS = {}
LAST_RESULT = None


def _emit(nc: bass.Bass, adjq, adjq7, s_own_q, s_perm_q, s_inT, s_outT):
    with tile.TileContext(nc) as tc:
        with (
            tc.tile_pool(name="raw", bufs=JC - 1) as raw_pool,
            tc.tile_pool(name="raw7", bufs=2) as raw7_pool,
            tc.tile_pool(name="work", bufs=1) as work,
            tc.tile_pool(name="singles", bufs=1) as singles,
            tc.tile_pool(name="psT", bufs=1, space="PSUM") as psT_pool,
            tc.tile_pool(name="psSin", bufs=1, space="PSUM") as psSin_pool,
            tc.tile_pool(name="psOut", bufs=1, space="PSUM") as psOut_pool,
            tc.tile_pool(name="psWarm", bufs=1, space="PSUM") as psWarm_pool,
        ):
            # ---- persistent tiles / gpsimd-side small loads -----------------
            wtile = singles.tile([P, 640], BF16, name="wtile")
            nc.gpsimd.memset(wtile, 0)

            # ---- input DMAs -------------------------------------------------
            # chunk loads on the sync HWDGE queue, issued up front
            raws = []
            for jc in range(JC - 1):
                r = raw_pool.tile([P, 2, IT, WJ], BF16, tag="raw")
                nc.sync.dma_start(out=r, in_=adjq[jc])
                raws.append(r)
                if jc == 0:
                    # small loads ride the gpsimd queue in parallel
                    ident_dram = nc.inline_tensor(
                        np.eye(P, dtype=np.float32), name="ident_const"
                    )
                    ident = singles.tile([P, P], F32)
                    nc.gpsimd.dma_start(out=ident, in_=ident_dram.ap())
                    s_own_sb = singles.tile([P, IT, D], BF16)
                    nc.gpsimd.dma_start(out=s_own_sb, in_=s_own_q)
                    s_perm_sb = singles.tile([P, JC * 2 * 2, D], BF16)
                    nc.gpsimd.dma_start(out=s_perm_sb, in_=s_perm_q)
            raw7 = [None, None]
            for sub in range(2):
                r = raw7_pool.tile([P, 2, IT, HJ], BF16, tag="raw7")
                nc.sync.dma_start(out=r, in_=adjq7[sub])
                raw7[sub] = r

            a_chs = [
                work.tile([P, IT, WJ], BF16, name=f"a_ch_{par}") for par in range(2)
            ]
            # aT[t] viewed three ways: f32 (evac), [512,2] bf16 (matmul rhs)
            aTs = [
                [work.tile([P, WJ, 2], BF16, name=f"aT_{par}_{t}") for t in range(2)]
                for par in range(2)
            ]
            sin_sb = work.tile([D, N], BF16, name="sin_sb")
            sout_sb = work.tile([D, RB], BF16, name="sout_sb")

            psT = [
                [psT_pool.tile([P, RB], F32, name=f"psT_{par}_{t}") for t in range(2)]
                for par in range(2)
            ]
            psum_sins = [
                psSin_pool.tile([D, WJ], F32, name=f"psum_sin_{par}")
                for par in range(2)
            ]
            psum_out = psOut_pool.tile([D, RB], F32)
            warm_ps = psWarm_pool.tile([P, WJ], F32)

            # ---- PE warm-up: defeat the HAM cold clock ----------------------
            for _ in range(N_WARM):
                nc.tensor.matmul(
                    warm_ps, lhsT=wtile[:, :P], rhs=wtile[:, P:P + WJ],
                    start=True, stop=True,
                )

            def emit_sout_mms(jc, t):
                """s_out accumulation for chunk jc, pair-block t (aT already
                evacuated)."""
                aT_b = aTs[jc % 2][t]
                for r in range(2):
                    k = (jc * 2 + t) * 2 + r
                    nc.tensor.matmul(
                        psum_out,
                        lhsT=s_perm_sb[:, k, :],
                        rhs=aT_b[:, :, r],
                        start=(k == 0),
                        stop=(k == 2 * 2 * JC - 1),
                    )

            # ---- main loop over full chunks 0..6 ----------------------------
            for jc in range(JC - 1):
                par = jc % 2
                a_ch = a_chs[par]
                nc.vector.tensor_add(
                    out=a_ch, in0=raws[jc][:, 0], in1=raws[jc][:, 1]
                )
                a_f32 = a_ch.bitcast(F32)  # [P, IT, WJ//2] packed bf16 pairs
                for t in range(2):
                    for it in range(IT):
                        nc.tensor.transpose(
                            psT[par][t][:, it * P:(it + 1) * P],
                            a_f32[:, it, t * P:(t + 1) * P],
                            ident,
                        )
                for it in range(IT):
                    nc.tensor.matmul(
                        psum_sins[par],
                        lhsT=s_own_sb[:, it, :],
                        rhs=a_ch[:, it, :],
                        start=(it == 0),
                        stop=(it == IT - 1),
                    )
                if jc > 0:
                    for t in range(2):
                        emit_sout_mms(jc - 1, t)
                for t in range(2):
                    nc.scalar.copy(out=aTs[par][t].bitcast(F32), in_=psT[par][t])
                nc.vector.tensor_copy(
                    out=sin_sb[:, jc * WJ:(jc + 1) * WJ], in_=psum_sins[par]
                )
                if jc % 2 == 1:
                    q = jc // 2
                    nc.gpsimd.dma_start(
                        out=s_inT[q], in_=sin_sb[:, q * (N // 4):(q + 1) * (N // 4)]
                    )

            # ---- last chunk: two 256-wide subchunks for a short tail --------
            jc = JC - 1
            par = jc % 2
            a_ch = a_chs[par]
            a_f32 = a_ch.bitcast(F32)
            for sub in range(2):  # sub == pair-block t
                nc.vector.tensor_add(
                    out=a_ch[:, :, sub * HJ:(sub + 1) * HJ],
                    in0=raw7[sub][:, 0],
                    in1=raw7[sub][:, 1],
                )
                for it in range(IT):
                    nc.tensor.transpose(
                        psT[par][sub][:, it * P:(it + 1) * P],
                        a_f32[:, it, sub * P:(sub + 1) * P],
                        ident,
                    )
                for it in range(IT):
                    nc.tensor.matmul(
                        psum_sins[par][:, sub * HJ:(sub + 1) * HJ],
                        lhsT=s_own_sb[:, it, :],
                        rhs=a_ch[:, it, sub * HJ:(sub + 1) * HJ],
                        start=(it == 0),
                        stop=(it == IT - 1),
                    )
                if sub == 0:
                    for t in range(2):
                        emit_sout_mms(jc - 1, t)
                nc.scalar.copy(out=aTs[par][sub].bitcast(F32), in_=psT[par][sub])
                emit_sout_mms(jc, sub)
            nc.vector.tensor_copy(
                out=sin_sb[:, jc * WJ:(jc + 1) * WJ], in_=psum_sins[par]
            )
            nc.gpsimd.dma_start(
                out=s_inT[3], in_=sin_sb[:, 3 * (N // 4):]
            )
            nc.vector.tensor_copy(out=sout_sb, in_=psum_out)
            nc.gpsimd.dma_start(out=s_outT, in_=sout_sb)


_ENGINE_SEM_PREFIX = {
    "PE": "PE_",
    "DVE": "DVE_",
    "Activation": "Activation_",
    "Pool": "Pool_",
    "SP": "SP_",
}

_SKIP_OPS = ("InstEventSemaphore", "InstDrain", "InstDMACopy", "InstBranch")


def _strip_self_waits(nc: bass.Bass) -> int:
    """Drop semaphore waits where an instruction waits on its OWN engine's
    completion semaphore.  Engine queues issue and complete in order, so such
    waits are always runtime-satisfied; Tile emits them anyway and they push
    instructions past walrus codegen's per-opcode sync-wait limits (most
    compute encodings accept a single wait)."""
    stripped = 0
    for _, inst in nc.inst_map.items():
        if type(inst).__name__ in _SKIP_OPS:
            continue
        si = getattr(inst, "sync_info", None)
        if si is None or not si.on_wait:
            continue
        eng = getattr(inst, "engine", None)
        prefix = _ENGINE_SEM_PREFIX.get(getattr(eng, "name", ""), None)
        if prefix is None:
            continue
        kept = [w for w in si.on_wait if not w.ant_name.startswith(prefix)]
        if len(kept) != len(si.on_wait):
            stripped += len(si.on_wait) - len(kept)
            si.on_wait = kept
    return stripped


def _build() -> bass.Bass:
    nc = bacc.Bacc("TRN2", num_devices=NCORES)
    adjq = nc.dram_tensor("adjq", [JC - 1, P, 2, IT, WJ], BF16, kind="ExternalInput")
    adjq7 = nc.dram_tensor("adjq7", [2, P, 2, IT, HJ], BF16, kind="ExternalInput")
    s_own_q = nc.dram_tensor("s_own_q", [P, IT, D], BF16, kind="ExternalInput")
    s_perm_q = nc.dram_tensor("s_perm_q", [P, JC * 2 * 2, D], BF16, kind="ExternalInput")
    s_inT = [
        nc.dram_tensor(f"s_inT_{h}", [D, N // 4], BF16, kind="ExternalOutput")
        for h in range(4)
    ]
    s_outT = nc.dram_tensor("s_outT", [D, RB], BF16, kind="ExternalOutput")
    _emit(
        nc,
        adjq.ap(),
        adjq7.ap(),
        s_own_q.ap(),
        s_perm_q.ap(),
        [t.ap() for t in s_inT],
        s_outT.ap(),
    )
    _strip_self_waits(nc)
    nc.finalize()
    return nc


_nc_cache = None


def _prep_core_inputs(adj_bf, s_bf, c):
    """Host-side relayout of core c's adjacency block and s tiles."""
    blk = adj_bf[c * RB:(c + 1) * RB]                    # (512, 4096, 2) bf16
    v = blk.reshape(IT, P, JC, WJ, 2)                     # it, p, jc, j, k
    v = v.transpose(2, 1, 4, 0, 3)                        # jc, p, k, it, j
    adjq = np.ascontiguousarray(v[: JC - 1])
    last = v[JC - 1]                                      # (p, k, it, 512)
    adjq7 = np.ascontiguousarray(
        last.reshape(P, 2, IT, 2, HJ).transpose(3, 0, 1, 2, 4)
    )                                                     # (sub, p, k, it, 256)
    s_own_q = np.ascontiguousarray(
        s_bf[c * RB:(c + 1) * RB].reshape(IT, P, D).transpose(1, 0, 2)
    )
    return {"adjq": adjq, "adjq7": adjq7, "s_own_q": s_own_q}


def kernel(adj: np.ndarray, s: np.ndarray):
    global _nc_cache, LAST_RESULT
    adj = np.asarray(adj)
    s = np.asarray(s)
    assert adj.shape == (N, N, 2) and s.shape == (N, D)

    if _nc_cache is None:
        _nc_cache = _build()
    nc = _nc_cache

    adj_bf = np.asarray(adj, np.float32).astype(ml_dtypes.bfloat16)
    s_bf = np.asarray(s, np.float32).astype(ml_dtypes.bfloat16)
    # s_perm[p, (jc, t, r)] = s[jc*512 + (t*128 + p)*2 + r]   (partition-major)
    s_perm = np.ascontiguousarray(
        s_bf.reshape(JC, 2, P, 2, D).transpose(2, 0, 1, 3, 4).reshape(P, JC * 4, D)
    )

    in_maps = []
    for c in range(NCORES):
        m = _prep_core_inputs(adj_bf, s_bf, c)
        m["s_perm_q"] = s_perm
        in_maps.append(m)

    res = bass_utils.run_bass_kernel_spmd(
        nc,
        in_maps,
        core_ids=list(range(NCORES)),
        trace=TRACE,
        **TRACE_KWARGS,
    )
    LAST_RESULT = res

    s_in = (
        np.sum(
            [
                np.concatenate(
                    [np.asarray(r[f"s_inT_{h}"], np.float32) for h in range(4)],
                    axis=1,
                )
                for r in res.results
            ],
            axis=0,
            dtype=np.float64,
        )
        .astype(np.float32)
        .T
    )
    s_out = np.concatenate(
        [np.asarray(r["s_outT"], np.float32).T for r in res.results], axis=0
    )
    return (np.ascontiguousarray(s_in), np.ascontiguousarray(s_out))
